# revision 6
# baseline (speedup 1.0000x reference)
"""LIF forward on 8 TRN2 cores — fused custom-DVE step + PE-packed spike output.

Recurrence (per element, scan over T):
    m_t = v_{t-1} * tau + x_t ; y_t = (m_t > v_th) ; v_t = m_t * (1 - y_t)

v_t <= v_th always, so one value z encodes the whole state:
    z_t = m_t if m_t <= v_th else 2.0      (custom DVE op, one per step)
    y_t = (z_t > v_th)

Spike readout is packed 4 channels -> 1 byte to cut HBM write traffic 4x:
    s_t = Sign(z_t - v_th) in {-1, 0, +1}           (ACT, bf16)
    packed[32g+j, n] = sum_r 4^r * s_{4G+g}[4j+r, n]  (PE matmul, PSUM accum
                       over the 4 steps g of group G; balanced base-4 digits
                       decode uniquely, |packed| <= 85 fits int8)
Host decodes digits and emits y = (digit == +1).

x: [T=16, B=32, C=128, H=32, W=32] f32, data-parallel over B (4 per core).
Host pre-transposes to [T, C, B_loc*S] so DMA is contiguous per partition.
"""

import sys

sys.path.insert(0, "/opt/trn_rl_repo")

from contextlib import ExitStack

import ml_dtypes
import numpy as np

import concourse.bass as bass
import concourse.tile as tile
from concourse import bacc, mybir
from concourse.bass_utils import run_bass_kernel_spmd
from concourse.dve_spec import Spec, Src0, Src1, C0, C1, C2, Zero, select, lower
from concourse.dve_uop import DveOpSpec
import concourse.dve_ops as _dve_ops
from concourse.dve_ops import DveOp

V_TH = 1.0
TAU = 0.5
SPIKE_Z = 2.0

T, B, C, H, W = 16, 32, 128, 32, 32
N_CORES = 8
B_LOC = B // N_CORES
S = H * W
FREE = B_LOC * S               # 4096

DT = mybir.dt.float32
BF = mybir.dt.bfloat16

N_CHUNKS = 2
CHUNK = FREE // N_CHUNKS       # 2048

N_GROUPS = T // 4              # 4 steps packed per PSUM accumulation group
MM_N = 512                     # matmul free-dim tile (one PSUM bank)


def _make_lif_op() -> DveOp:
    name = "LIF_STEP_FUSED_ANT"
    v = select(Src0 > C1, Zero, Src0)        # decode prev z -> v
    m = v * C0 + Src1                         # m = tau*v + x
    body = select(m > C1, C2, m)              # encode: spike -> imm2

    def ref(in0, in1, s0, s1, imm2):
        vv = np.where(in0 > s1, 0.0, in0)
        mm = vv * s0 + in1
        return np.where(mm > s1, imm2, mm).astype(np.float32)

    spec = Spec(body=body, reference=ref)
    shas = {
        ver: DveOpSpec(
            name=name, opcode=0, uops=lower(spec, ver=ver), rd1_en=True
        ).sha(ver)
        for ver in ("v3", "v4")
    }
    for o in _dve_ops.OPS:
        if o.name == name:
            return o
    op = DveOp(name, spec, subdim=False, uops_sha=shas)
    _dve_ops.OPS.append(op)
    _dve_ops._SUB_OPCODE_FOR_NAME[name] = (
        _dve_ops._CUSTOM_DVE_ROW_BASE + len(_dve_ops.OPS) - 1
    )
    _dve_ops.CUSTOM_DVE_SPECS[name] = spec
    return op


LIF_OP = _make_lif_op()


def _pack_weights() -> np.ndarray:
    """w[k, g*128 + m] = 4^(k%4) if m == 32g + k//4 else 0, bf16 [128, 512]."""
    w = np.zeros((128, 4, 128), np.float32)
    for k in range(128):
        for g in range(4):
            w[k, g, 32 * g + k // 4] = float(4 ** (k % 4))
    return w.reshape(128, 512).astype(ml_dtypes.bfloat16)


def build_kernel() -> bass.Bass:
    nc = bacc.Bacc(
        "TRN2", target_bir_lowering=False, debug=False, num_devices=N_CORES
    )
    x_d = nc.dram_tensor("x", [T, C, FREE], DT, kind="ExternalInput").ap()
    w_d = nc.dram_tensor("w", [128, 512], BF, kind="ExternalInput").ap()
    yp_d = nc.dram_tensor(
        "yp", [N_GROUPS, C, FREE], mybir.dt.int8, kind="ExternalOutput"
    ).ap()

    with ExitStack() as ctx:
        tc = ctx.enter_context(tile.TileContext(nc))
        w_pool = ctx.enter_context(tc.tile_pool(name="w", bufs=1))
        x_pool = ctx.enter_context(tc.tile_pool(name="x", bufs=6))
        z_pool = ctx.enter_context(tc.tile_pool(name="z", bufs=2 * N_CHUNKS))
        s_pool = ctx.enter_context(tc.tile_pool(name="s", bufs=3 * N_CHUNKS))
        o_pool = ctx.enter_context(tc.tile_pool(name="o", bufs=2))
        psum_pool = ctx.enter_context(
            tc.tile_pool(name="ps", bufs=1, space="PSUM")
        )

        # -V_TH bias const for ACT Sign: a pool tile memset inside the
        # TileContext (dependency-tracked; avoids a start-up all-engine
        # barrier before the first DMA trigger)
        cb = w_pool.tile([128, 1], DT, tag="cb", name="const_bias")
        nc.gpsimd.memset(cb[:], -V_TH)
        nc.const_aps.aps[(DT, -V_TH)] = cb[:]

        z_cur = []
        for k in range(N_CHUNKS):
            zt = z_pool.tile([C, CHUNK], DT, tag="z", name=f"z_init{k}")
            nc.gpsimd.memset(zt[:], 0.0)
            z_cur.append(zt)

        wt = w_pool.tile([128, 512], BF, tag="w")
        for G in range(N_GROUPS):
            ps = psum_pool.tile([128, FREE], DT, tag="ps")
            for g in range(4):
                t = 4 * G + g
                xt = x_pool.tile([C, FREE], DT, tag="x", name=f"x{t}")
                if t <= 1 or t >= T - 2:
                    # ramp/tail: land compute chunks piecewise; slot the
                    # (tiny) pack-weight load between the first pieces
                    nc.sync.dma_start(
                        out=xt[:, :CHUNK], in_=x_d[t, :, :CHUNK]
                    )
                    if t == 0:
                        nc.sync.dma_start(out=wt[:], in_=w_d)
                    nc.sync.dma_start(
                        out=xt[:, CHUNK:], in_=x_d[t, :, CHUNK:]
                    )
                else:
                    nc.sync.dma_start(out=xt[:], in_=x_d[t])
                # first/last steps: halve the op width so the chain starts
                # earlier (ramp) / drains earlier (tail)
                sub = 2 if t in (0, T - 2, T - 1) else 1
                for k in range(N_CHUNKS):
                    c0 = k * CHUNK
                    zn = z_pool.tile([C, CHUNK], DT, tag="z", name=f"z{t}_{k}")
                    st = s_pool.tile([C, CHUNK], BF, tag="s", name=f"s{t}_{k}")
                    for q in range(sub):
                        w0, w1 = q * CHUNK // sub, (q + 1) * CHUNK // sub
                        nc.vector._custom_dve(
                            LIF_OP, out=zn[:, w0:w1],
                            in0=z_cur[k][:, w0:w1],
                            in1=xt[:, c0 + w0 : c0 + w1],
                            s0=TAU, s1=V_TH, imm2=SPIKE_Z,
                        )
                        nc.scalar.activation(
                            st[:, w0:w1], zn[:, w0:w1],
                            mybir.ActivationFunctionType.Sign, bias=-V_TH,
                        )
                    z_cur[k] = zn
                    for j in range(CHUNK // MM_N):
                        f0 = c0 + j * MM_N
                        nc.tensor.matmul(
                            ps[:, f0 : f0 + MM_N],
                            wt[:, 128 * g : 128 * (g + 1)],
                            st[:, j * MM_N : (j + 1) * MM_N],
                            start=(g == 0),
                            stop=(g == 3),
                        )
            # pack PSUM -> int8 and store. Steady state: 2 of 3 group
            # copies on ACT, 1 on DVE (balances ACT ~79us vs DVE ~76us
            # busy). Last group: split 4-ways across DVE+ACT with half
            # stores for the shortest tail.
            ot = o_pool.tile([C, FREE], mybir.dt.int8, tag="o", name=f"o{G}")
            if G < N_GROUPS - 1:
                for k in range(N_CHUNKS):
                    c0 = k * CHUNK
                    if G == 1 and k == 0:
                        nc.vector.tensor_copy(
                            ot[:, c0 : c0 + CHUNK], ps[:, c0 : c0 + CHUNK]
                        )
                    else:
                        nc.scalar.activation(
                            ot[:, c0 : c0 + CHUNK], ps[:, c0 : c0 + CHUNK],
                            mybir.ActivationFunctionType.Copy,
                        )
                nc.scalar.dma_start(out=yp_d[G], in_=ot[:])
            else:
                q = FREE // 4
                for k in range(4):
                    c0 = k * q
                    if k % 2 == 0:
                        nc.vector.tensor_copy(
                            ot[:, c0 : c0 + q], ps[:, c0 : c0 + q]
                        )
                    else:
                        nc.scalar.activation(
                            ot[:, c0 : c0 + q], ps[:, c0 : c0 + q],
                            mybir.ActivationFunctionType.Copy,
                        )
                    if k % 2 == 1:
                        nc.scalar.dma_start(
                            out=yp_d[G, :, c0 - q : c0 + q],
                            in_=ot[:, c0 - q : c0 + q],
                        )
    nc.finalize()
    return nc


_NC_CACHE = None


def _get_nc():
    global _NC_CACHE
    if _NC_CACHE is None:
        _NC_CACHE = build_kernel()
    return _NC_CACHE


def _shard_inputs(x: np.ndarray) -> list[dict]:
    xf = np.asarray(x, dtype=np.float32).reshape(T, B, C, S)
    w = _pack_weights()
    maps = []
    for k in range(N_CORES):
        xs = xf[:, k * B_LOC : (k + 1) * B_LOC]          # [T, B_loc, C, S]
        xs = np.ascontiguousarray(xs.transpose(0, 2, 1, 3))  # [T, C, B_loc, S]
        maps.append({"x": xs.reshape(T, C, FREE), "w": w})
    return maps


def _decode_packed(yp: np.ndarray) -> np.ndarray:
    """yp [N_GROUPS, 128, FREE] int8 -> y [T, C, FREE] f32 in {0,1}."""
    v = yp.reshape(N_GROUPS, 4, 32, FREE).astype(np.int32)  # [G, g, j, n]
    digits = []
    for r in (3, 2, 1, 0):
        # balanced decode: remainder after removing higher digits is within
        # (-p/2, p/2), so floor((v + p/2)/p) is the digit
        p = 4**r
        d = (v + p // 2) // p
        digits.append(d)
        v = v - d * p
    # digits[idx] is r=3,2,1,0 -> reorder to r=0..3
    digits = digits[::-1]
    y = np.empty((N_GROUPS, 4, 128, FREE), np.float32)  # [G, g, c, n]
    for r in range(4):
        y[:, :, r::4, :] = (digits[r] == 1).astype(np.float32)
    return y.reshape(T, C, FREE)


def _unshard_output(results) -> np.ndarray:
    outs = []
    for k in range(N_CORES):
        yk = _decode_packed(results[k]["yp"]).reshape(T, C, B_LOC, S)
        outs.append(yk.transpose(0, 2, 1, 3))            # [T, B_loc, C, S]
    return np.concatenate(outs, axis=1)                  # [T, B, C, S]


def kernel(x: np.ndarray) -> np.ndarray:
    assert x.shape == (T, B, C, H, W), x.shape
    in_dtype = x.dtype
    nc = _get_nc()
    res = run_bass_kernel_spmd(nc, _shard_inputs(x), list(range(N_CORES)))
    out = _unshard_output(res.results)
    return out.reshape(T, B, C, H, W).astype(in_dtype, copy=False)


if __name__ == "__main__":
    x = np.random.randn(T, B, C, H, W).astype(np.float32)
    y = kernel(x)
    print("out", y.shape, y.dtype, "spike rate", y.mean())


# revision 7
# speedup vs baseline: 1.1532x; 1.1532x over previous
"""LIF forward on 8 TRN2 cores — fused custom-DVE step + PE-packed spike output.

Recurrence (per element, scan over T):
    m_t = v_{t-1} * tau + x_t ; y_t = (m_t > v_th) ; v_t = m_t * (1 - y_t)

v_t <= v_th always, so one value z encodes the whole state:
    z_t = m_t if m_t <= v_th else 2.0      (custom DVE op, one per step)
    y_t = (z_t > v_th)

Spike readout is packed 4 channels -> 1 byte to cut HBM write traffic 4x:
    s_t = Sign(z_t - v_th) in {-1, 0, +1}           (ACT, bf16)
    packed[32g+j, n] = sum_r 4^r * s_{4G+g}[4j+r, n]  (PE matmul, PSUM accum
                       over the 4 steps g of group G; balanced base-4 digits
                       decode uniquely, |packed| <= 85 fits int8)
Host decodes digits and emits y = (digit == +1).

x: [T=16, B=32, C=128, H=32, W=32] f32, data-parallel over B (4 per core).
Host pre-transposes to [T, C, B_loc*S] so DMA is contiguous per partition.
"""

import sys

sys.path.insert(0, "/opt/trn_rl_repo")

from contextlib import ExitStack

import ml_dtypes
import numpy as np

import concourse.bass as bass
import concourse.tile as tile
from concourse import bacc, mybir
from concourse.bass_utils import run_bass_kernel_spmd
from concourse.dve_spec import Spec, Src0, Src1, C0, C1, C2, Zero, select, lower
from concourse.dve_uop import DveOpSpec
import concourse.dve_ops as _dve_ops
from concourse.dve_ops import DveOp

V_TH = 1.0
TAU = 0.5
SPIKE_Z = 2.0

T, B, C, H, W = 16, 32, 128, 32, 32
N_CORES = 8
B_LOC = B // N_CORES
S = H * W
FREE = B_LOC * S               # 4096

DT = mybir.dt.float32
BF = mybir.dt.bfloat16

N_CHUNKS = 2
CHUNK = FREE // N_CHUNKS       # 2048

N_GROUPS = T // 4              # 4 steps packed per PSUM accumulation group
MM_N = 512                     # matmul free-dim tile (one PSUM bank)


def _make_lif_op() -> DveOp:
    name = "LIF_STEP_FUSED_ANT"
    v = select(Src0 > C1, Zero, Src0)        # decode prev z -> v
    m = v * C0 + Src1                         # m = tau*v + x
    body = select(m > C1, C2, m)              # encode: spike -> imm2

    def ref(in0, in1, s0, s1, imm2):
        vv = np.where(in0 > s1, 0.0, in0)
        mm = vv * s0 + in1
        return np.where(mm > s1, imm2, mm).astype(np.float32)

    spec = Spec(body=body, reference=ref)
    shas = {
        ver: DveOpSpec(
            name=name, opcode=0, uops=lower(spec, ver=ver), rd1_en=True
        ).sha(ver)
        for ver in ("v3", "v4")
    }
    for o in _dve_ops.OPS:
        if o.name == name:
            return o
    op = DveOp(name, spec, subdim=False, uops_sha=shas)
    _dve_ops.OPS.append(op)
    _dve_ops._SUB_OPCODE_FOR_NAME[name] = (
        _dve_ops._CUSTOM_DVE_ROW_BASE + len(_dve_ops.OPS) - 1
    )
    _dve_ops.CUSTOM_DVE_SPECS[name] = spec
    return op


LIF_OP = _make_lif_op()


def _pack_weights() -> np.ndarray:
    """w[k, g*128 + m] = 4^(k%4) if m == 32g + k//4 else 0, bf16 [128, 512]."""
    w = np.zeros((128, 4, 128), np.float32)
    for k in range(128):
        for g in range(4):
            w[k, g, 32 * g + k // 4] = float(4 ** (k % 4))
    return w.reshape(128, 512).astype(ml_dtypes.bfloat16)


def build_kernel() -> bass.Bass:
    nc = bacc.Bacc(
        "TRN2", target_bir_lowering=False, debug=False, num_devices=N_CORES
    )
    x_d = nc.dram_tensor("x", [T, C, FREE], DT, kind="ExternalInput").ap()
    w_d = nc.dram_tensor("w", [128, 512], BF, kind="ExternalInput").ap()
    yp_d = nc.dram_tensor(
        "yp", [N_GROUPS, C, FREE], mybir.dt.int8, kind="ExternalOutput"
    ).ap()

    with ExitStack() as ctx:
        tc = ctx.enter_context(tile.TileContext(nc))
        w_pool = ctx.enter_context(tc.tile_pool(name="w", bufs=1))
        x_pool = ctx.enter_context(tc.tile_pool(name="x", bufs=6))
        z_pool = ctx.enter_context(tc.tile_pool(name="z", bufs=2 * N_CHUNKS))
        s_pool = ctx.enter_context(tc.tile_pool(name="s", bufs=3 * N_CHUNKS))
        o_pool = ctx.enter_context(tc.tile_pool(name="o", bufs=2))
        psum_pool = ctx.enter_context(
            tc.tile_pool(name="ps", bufs=1, space="PSUM")
        )

        # -V_TH bias const for ACT Sign: a pool tile memset inside the
        # TileContext (dependency-tracked; avoids a start-up all-engine
        # barrier before the first DMA trigger)
        cb = w_pool.tile([128, 1], DT, tag="cb", name="const_bias")
        nc.gpsimd.memset(cb[:], -V_TH)
        nc.const_aps.aps[(DT, -V_TH)] = cb[:]

        z_cur = []
        for k in range(N_CHUNKS):
            zt = z_pool.tile([C, CHUNK], DT, tag="z", name=f"z_init{k}")
            nc.gpsimd.memset(zt[:], 0.0)
            z_cur.append(zt)

        wt = w_pool.tile([128, 512], BF, tag="w")
        for G in range(N_GROUPS):
            ps = psum_pool.tile([128, FREE], DT, tag="ps")
            for g in range(4):
                t = 4 * G + g
                xt = x_pool.tile([C, FREE], DT, tag="x", name=f"x{t}")
                if t <= 1 or t >= T - 2:
                    # ramp/tail: land compute chunks piecewise
                    nc.sync.dma_start(
                        out=xt[:, :CHUNK], in_=x_d[t, :, :CHUNK]
                    )
                    if t == 0:
                        # descriptor-heavy 128 KB weight load rides the
                        # (currently idle) scalar ring, off the x stream
                        nc.scalar.dma_start(out=wt[:], in_=w_d)
                    nc.sync.dma_start(
                        out=xt[:, CHUNK:], in_=x_d[t, :, CHUNK:]
                    )
                else:
                    nc.sync.dma_start(out=xt[:], in_=x_d[t])
                # first/last steps: halve the op width so the chain starts
                # earlier (ramp) / drains earlier (tail)
                sub = 2 if t in (0, T - 2, T - 1) else 1
                for k in range(N_CHUNKS):
                    c0 = k * CHUNK
                    zn = z_pool.tile([C, CHUNK], DT, tag="z", name=f"z{t}_{k}")
                    st = s_pool.tile([C, CHUNK], BF, tag="s", name=f"s{t}_{k}")
                    for q in range(sub):
                        w0, w1 = q * CHUNK // sub, (q + 1) * CHUNK // sub
                        nc.vector._custom_dve(
                            LIF_OP, out=zn[:, w0:w1],
                            in0=z_cur[k][:, w0:w1],
                            in1=xt[:, c0 + w0 : c0 + w1],
                            s0=TAU, s1=V_TH, imm2=SPIKE_Z,
                        )
                        nc.scalar.activation(
                            st[:, w0:w1], zn[:, w0:w1],
                            mybir.ActivationFunctionType.Sign, bias=-V_TH,
                        )
                    z_cur[k] = zn
                    for j in range(CHUNK // MM_N):
                        f0 = c0 + j * MM_N
                        nc.tensor.matmul(
                            ps[:, f0 : f0 + MM_N],
                            wt[:, 128 * g : 128 * (g + 1)],
                            st[:, j * MM_N : (j + 1) * MM_N],
                            start=(g == 0),
                            stop=(g == 3),
                        )
            # pack PSUM -> int8 and store. Steady state: 2 of 3 group
            # copies on ACT, 1 on DVE (balances ACT ~79us vs DVE ~76us
            # busy). Last group: split 4-ways across DVE+ACT with half
            # stores for the shortest tail.
            ot = o_pool.tile([C, FREE], mybir.dt.int8, tag="o", name=f"o{G}")
            if G < N_GROUPS - 1:
                for k in range(N_CHUNKS):
                    c0 = k * CHUNK
                    if G == 1 and k == 0:
                        nc.vector.tensor_copy(
                            ot[:, c0 : c0 + CHUNK], ps[:, c0 : c0 + CHUNK]
                        )
                    else:
                        nc.scalar.activation(
                            ot[:, c0 : c0 + CHUNK], ps[:, c0 : c0 + CHUNK],
                            mybir.ActivationFunctionType.Copy,
                        )
                nc.scalar.dma_start(out=yp_d[G], in_=ot[:])
            else:
                q = FREE // 4
                for k in range(4):
                    c0 = k * q
                    if k % 2 == 0:
                        nc.vector.tensor_copy(
                            ot[:, c0 : c0 + q], ps[:, c0 : c0 + q]
                        )
                    else:
                        nc.scalar.activation(
                            ot[:, c0 : c0 + q], ps[:, c0 : c0 + q],
                            mybir.ActivationFunctionType.Copy,
                        )
                    if k % 2 == 1:
                        nc.scalar.dma_start(
                            out=yp_d[G, :, c0 - q : c0 + q],
                            in_=ot[:, c0 - q : c0 + q],
                        )
    nc.finalize()
    return nc


_NC_CACHE = None


def _get_nc():
    global _NC_CACHE
    if _NC_CACHE is None:
        _NC_CACHE = build_kernel()
    return _NC_CACHE


def _shard_inputs(x: np.ndarray) -> list[dict]:
    xf = np.asarray(x, dtype=np.float32).reshape(T, B, C, S)
    w = _pack_weights()
    maps = []
    for k in range(N_CORES):
        xs = xf[:, k * B_LOC : (k + 1) * B_LOC]          # [T, B_loc, C, S]
        xs = np.ascontiguousarray(xs.transpose(0, 2, 1, 3))  # [T, C, B_loc, S]
        maps.append({"x": xs.reshape(T, C, FREE), "w": w})
    return maps


def _decode_packed(yp: np.ndarray) -> np.ndarray:
    """yp [N_GROUPS, 128, FREE] int8 -> y [T, C, FREE] f32 in {0,1}."""
    v = yp.reshape(N_GROUPS, 4, 32, FREE).astype(np.int32)  # [G, g, j, n]
    digits = []
    for r in (3, 2, 1, 0):
        # balanced decode: remainder after removing higher digits is within
        # (-p/2, p/2), so floor((v + p/2)/p) is the digit
        p = 4**r
        d = (v + p // 2) // p
        digits.append(d)
        v = v - d * p
    # digits[idx] is r=3,2,1,0 -> reorder to r=0..3
    digits = digits[::-1]
    y = np.empty((N_GROUPS, 4, 128, FREE), np.float32)  # [G, g, c, n]
    for r in range(4):
        y[:, :, r::4, :] = (digits[r] == 1).astype(np.float32)
    return y.reshape(T, C, FREE)


def _unshard_output(results) -> np.ndarray:
    outs = []
    for k in range(N_CORES):
        yk = _decode_packed(results[k]["yp"]).reshape(T, C, B_LOC, S)
        outs.append(yk.transpose(0, 2, 1, 3))            # [T, B_loc, C, S]
    return np.concatenate(outs, axis=1)                  # [T, B, C, S]


def kernel(x: np.ndarray) -> np.ndarray:
    assert x.shape == (T, B, C, H, W), x.shape
    in_dtype = x.dtype
    nc = _get_nc()
    res = run_bass_kernel_spmd(nc, _shard_inputs(x), list(range(N_CORES)))
    out = _unshard_output(res.results)
    return out.reshape(T, B, C, H, W).astype(in_dtype, copy=False)


if __name__ == "__main__":
    x = np.random.randn(T, B, C, H, W).astype(np.float32)
    y = kernel(x)
    print("out", y.shape, y.dtype, "spike rate", y.mean())
